# revision 24
# baseline (speedup 1.0000x reference)
"""Trainium2 Bass kernel for an 8-layer dense MLP (784->512x6->10) + softmax.

Strategy (hardcoded for batch=65536, 8 NeuronCores, pure data parallel):
  - Each core handles 8192 rows of the batch; weights replicated.
  - All matmuls run in fp8-e4m3 with perf_mode=DoubleRow: the PE holds two
    fp8 weight planes per cell, contracting 256 features per matmul at
    2 MACs/cell/cycle (~2x the fp32r column rate). PSUM accumulates fp32.
  - Per-tensor power-of-2 scales keep fp8 ranges centered: weights scaled by
    beta_l, activations by alpha_l (estimated from a host-side subsample
    forward pass). The rescale alpha_l/(alpha_{l-1} beta_l) and the bias are
    folded into the PSUM-draining op.
  - PSUM drains are split across engines to balance throughput (ACT does a
    [128,512] relu-drain in ~720ns, the DVE pair mult-add + relu-cast in
    ~1.1us, and the PE issues a DoubleRow matmul every ~216ns): 19 of the 28
    per-tile drains go to ACT, 9 to DVE; all 12 dropout mask-multiplies go
    to GpSimd.
  - Dropout masks (jax threefry, key 42) are bit-exactly precomputed on host
    and shipped as {0,1} uint8 masks; 1/(1-p) is folded into the next layer's
    weights before quantization.
  - Softmax: exp on ACT (bias=b8, scale=1/(alpha7 beta8)), class sums via a
    10x10 ones-matmul on the PE (replicated across the 10 partitions in
    PSUM), reciprocal + multiply on DVE. No max-subtraction (logits are
    O(0.1); exp is safe in fp32).
"""

import numpy as np

BATCH = 65536
D_IN = 784
D_PAD = 1024               # padded input features: 4 DoubleRow groups of 256
KG1 = 4                    # layer-1 DoubleRow k-groups
H = 512
KO = 4                     # 128-feature chunks per hidden activation
KG = 2                     # hidden-layer DoubleRow k-groups (2 x 256 = 512)
C = 10
CP = 16                    # layer-8 output padded to 16 for AP alignment
N_CORES = 8
B_CORE = BATCH // N_CORES  # 8192
BT = 512                   # batch tile (matmul moving free dim)

DROP_LAYERS = (2, 4, 6)    # dropout applied to these layers' outputs
KEEP = {2: 0.8, 4: 0.7, 6: 0.5}

# Drain-engine split: chunk n of layer l drains on DVE iff listed here.
# (Measured: ACT does a [128,512] drain in one ~720ns op; a DVE drain needs
# two tensor_scalar ops at ~1.9us total. ACT saturates at ~22 drains/tile,
# so 6 late chunks of the non-dropout layers go to DVE; the pair-wavefront
# below hides their latency behind the partner tile's matmuls.)
DVE_DRAINS = {(1, 2), (1, 3), (3, 2), (3, 3), (5, 3), (7, 3)}


def build_bass(b_core: int, act_scale: dict):
    """Build the Bass module for one core processing b_core batch rows.

    act_scale maps layer -> float scale applied inside the PSUM-draining
    op (alpha_l / (alpha_{l-1} beta_l) for hidden, 1/(alpha7 beta8) for
    the logits).
    """
    import concourse.mybir as mybir
    import concourse.tile as tile
    from concourse import bacc

    f32 = mybir.dt.float32
    f32r = mybir.dt.float32r
    f8 = mybir.dt.float8e4
    u8 = mybir.dt.uint8
    AF = mybir.ActivationFunctionType
    ALU = mybir.AluOpType
    DR = mybir.MatmulPerfMode.DoubleRow

    nbt = b_core // BT

    nc = bacc.Bacc("TRN2", target_bir_lowering=False, debug=False)

    xT = nc.dram_tensor("xT", [128, nbt, 2 * KG1, BT], f8, kind="ExternalInput")
    w_h = {1: nc.dram_tensor("w1", [128, KG1, 2, H], f8, kind="ExternalInput")}
    for l in range(2, 8):
        w_h[l] = nc.dram_tensor(f"w{l}", [128, KG, 2, H], f8, kind="ExternalInput")
    w8_h = nc.dram_tensor("w8", [128, KG, 2, CP], f8, kind="ExternalInput")
    bias17_h = nc.dram_tensor("bias17", [128, 28], f32, kind="ExternalInput")
    b8c_h = nc.dram_tensor("b8c", [128, 1], f32, kind="ExternalInput")
    m_h = {
        l: nc.dram_tensor(f"m{l}", [128, nbt, KO, BT], u8, kind="ExternalInput")
        for l in DROP_LAYERS
    }
    y_h = nc.dram_tensor("yT", [C, b_core], f32, kind="ExternalOutput")

    with tile.TileContext(nc) as tc:
        with (
            tc.tile_pool(name="wpool", bufs=1) as wpool,
            tc.tile_pool(name="xpool", bufs=4) as xpool,
            tc.tile_pool(name="hpool", bufs=5) as hpool,
            tc.tile_pool(name="tpool", bufs=3) as tpool,
            tc.tile_pool(name="mpool", bufs=4) as mpool,
            tc.tile_pool(name="spool", bufs=4) as spool,
            tc.tile_pool(name="psum", bufs=4, space="PSUM") as pp,
            tc.tile_pool(name="psum8", bufs=2, space="PSUM") as pp8,
            tc.tile_pool(name="psums", bufs=2, space="PSUM") as pps,
        ):
            gate = {"inst": None}
            chain = {"prev": None}

            def chained(di):
                if chain["prev"] is not None:
                    tile.add_dep_helper(di.ins, chain["prev"].ins, sync=True)
                chain["prev"] = di
                return di

            def load_bt(bt, in_chain=False):
                xt = xpool.tile([128, 2 * KG1, BT], f8, tag="xt", name="xt")
                di = nc.sync.dma_start(xt[:], xT.ap()[:, bt, :, :])
                if in_chain:
                    chained(di)
                if gate["inst"] is not None:
                    # Prefetches for bt>=2 may not be hoisted ahead of the
                    # weight stream: gate them on the last hidden weight DMA.
                    tile.add_dep_helper(di.ins, gate["inst"], sync=True)
                mt = {}
                for l in DROP_LAYERS:
                    mt[l] = mpool.tile([128, KO, BT], u8, tag=f"m{l}", name=f"m{l}_t")
                    mi = nc.sync.dma_start(mt[l][:], m_h[l].ap()[:, bt, :, :])
                    if gate["inst"] is not None:
                        tile.add_dep_helper(mi.ins, gate["inst"], sync=True)
                return xt, mt

            # Warm the PE HAM clock-gate with dummy fp32 matmuls that run
            # during the initial DMA wait (~3.4us of activity -> K=8/8).
            warm_w = wpool.tile([128, 128], f32, tag="warm_w")
            warm_x = wpool.tile([128, BT], f32, tag="warm_x")
            nc.vector.memset(warm_w[:], 0)
            nc.vector.memset(warm_x[:], 0)
            ones10 = wpool.tile([C, C], f32, tag="ones10")
            nc.vector.memset(ones10[:], 1.0)
            warm_ps = pp.tile([128, BT], f32, tag="ps", name="warm_ps")
            for _ in range(7):
                nc.tensor.matmul(warm_ps[:], lhsT=warm_w[:], rhs=warm_x[:])

            # Startup DMAs are chained into a forced serial order
            # xt0 -> w1 -> xt1 -> w2..w7 so each transfer gets the full queue
            # bandwidth and the scheduler cannot hoist prefetches ahead of the
            # weight stream; the two-tile wavefront below consumes them in
            # exactly this order.
            xt0, mt0 = load_bt(0, in_chain=True)
            w_t = {1: wpool.tile([128, KG1, 2, H], f8, tag="w1", name="w1_t")}
            chained(nc.sync.dma_start(w_t[1][:], w_h[1].ap()))
            xt1, mt1 = load_bt(1, in_chain=True)
            w7_dma = None
            for l in range(2, 8):
                w_t[l] = wpool.tile([128, KG, 2, H], f8, tag=f"w{l}", name=f"w{l}_t")
                w7_dma = chained(nc.sync.dma_start(w_t[l][:], w_h[l].ap()))
            w8_t = wpool.tile([128, KG, 2, CP], f8, tag="w8")
            nc.sync.dma_start(w8_t[:], w8_h.ap())
            bias17_t = wpool.tile([128, 28], f32, tag="bias17")
            nc.sync.dma_start(bias17_t[:], bias17_h.ap())
            b8c_t = wpool.tile([128, 1], f32, tag="b8c")
            nc.sync.dma_start(b8c_t[:], b8c_h.ap())
            gate["inst"] = w7_dma.ins

            def hidden_layer(l, src, mt):
                kg_in = KG1 if l == 1 else KG
                hn = hpool.tile([128, KO, BT], f8, tag="h", name="h")
                # kg-outer emission: all kg=0 matmuls for the 4 n-chunks
                # first, then kg=1, ... — the chunks feeding the NEXT
                # layer's kg=1 groups are thereby needed ~900ns later,
                # hiding drain/mask latency.
                pss = [pp.tile([128, BT], f32, tag="ps", name="ps") for _ in range(KO)]
                for kg in range(kg_in):
                    for n in range(KO):
                        nc.tensor.matmul(
                            pss[n][:],
                            lhsT=w_t[l][:, kg, :, n * 128 : (n + 1) * 128],
                            rhs=src[:, 2 * kg : 2 * kg + 2, :],
                            start=(kg == 0),
                            stop=(kg == kg_in - 1),
                            perf_mode=DR,
                        )
                for n in range(KO):
                    ps = pss[n]
                    bias_ap = bias17_t[:, (l - 1) * 4 + n : (l - 1) * 4 + n + 1]
                    if (l, n) in DVE_DRAINS:
                        # two-op DVE drain: (scale*ps + bias) then relu+fp8
                        t32 = tpool.tile([128, BT], f32, tag="t32", name="t32")
                        nc.vector.tensor_scalar(
                            t32[:], ps[:], act_scale[l], scalar2=bias_ap,
                            op0=ALU.mult, op1=ALU.add,
                        )
                        nc.vector.tensor_scalar(
                            hn[:, n, :], t32[:], 0.0, scalar2=None, op0=ALU.max,
                        )
                    else:
                        # relu(scale*psum + bias) fused, PSUM -> fp8 SBUF
                        nc.scalar.activation(
                            hn[:, n, :], ps[:], AF.Relu,
                            bias=bias_ap, scale=act_scale[l],
                        )
                    if l in DROP_LAYERS:
                        # gpsimd's tensor_tensor is slow/erratic in situ
                        # (1.4-3.8us even with slack) — all masks on DVE.
                        nc.vector.tensor_tensor(
                            hn[:, n, :], hn[:, n, :], mt[l][:, n, :], ALU.mult
                        )
                return hn

            ot_all = wpool.tile([C, b_core], f32, tag="ot_all")

            def final_logits(h):
                # layer 8 (512->10 padded to 16), feature-major out, then
                # exp (bias=b8, scale=1/(a7 b8)) on ACT.
                ps8 = pp8.tile([CP, BT], f32, tag="ps8", name="ps8")
                for kg in range(KG):
                    nc.tensor.matmul(
                        ps8[:],
                        lhsT=w8_t[:, kg, :, :],
                        rhs=h[:, 2 * kg : 2 * kg + 2, :],
                        start=(kg == 0),
                        stop=(kg == KG - 1),
                        perf_mode=DR,
                    )
                ex = spool.tile([C, BT], f32r, tag="ex", name="ex")
                nc.scalar.activation(
                    ex[:], ps8[:C, :], AF.Exp, bias=b8c_t[:C, 0:1],
                    scale=act_scale[8],
                )
                return ex

            def final_tail(ex, bs):
                # Softmax tail, deferred one pair so the 3.3us reciprocals
                # land in the DVE queue behind the NEXT pair's drains instead
                # of ahead of them: class sums replicated via a 10x10
                # ones-matmul on PE, reciprocal + multiply on DVE.
                ps_s = pps.tile([C, BT], f32, tag="ps_s", name="ps_s")
                nc.tensor.matmul(ps_s[:], lhsT=ones10[:].bitcast(f32r), rhs=ex[:])
                rsum = spool.tile([C, BT], f32, tag="rsum", name="rsum")
                nc.vector.reciprocal(rsum[:], ps_s[:])
                # Accumulate normalized outputs in SBUF; one DMA at the end
                # so no per-tile queue ever blocks on the softmax tail.
                nc.vector.tensor_tensor(
                    ot_all[:, bs : bs + BT], ex[:], rsum[:], ALU.mult
                )

            # All tiles run as two-tile wavefront pairs: while tile A's layer
            # drains complete on ACT/DVE, the PE runs tile B's matmuls for
            # the same layer, hiding drain latency. Pair 0 additionally
            # covers the startup weight-stream window.
            pending = None
            preloaded = None
            for pair in range(0, nbt, 2):
                if pair == 0:
                    curA, mtA, curB, mtB = xt0, mt0, xt1, mt1
                else:
                    curA, mtA, curB, mtB = preloaded
                for l in range(1, 8):
                    curA = hidden_layer(l, curA, mtA)
                    curB = hidden_layer(l, curB, mtB)
                    if l == 2 and pair + 2 < nbt:
                        # Prefetch the next pair's x/mask tiles a full pair
                        # ahead so L1 never waits on the inbound DMA.
                        nxA, nmA = load_bt(pair + 2)
                        nxB, nmB = load_bt(pair + 3)
                        preloaded = (nxA, nmA, nxB, nmB)
                if pending is not None:
                    final_tail(pending[0], pending[1])
                    final_tail(pending[2], pending[3])
                pending = (
                    final_logits(curA), pair * BT,
                    final_logits(curB), (pair + 1) * BT,
                )
            final_tail(pending[0], pending[1])
            final_tail(pending[2], pending[3])

            nc.sync.dma_start(y_h.ap(), ot_all[:])

    nc.compile()
    return nc


def _pow2_scale(maxabs: float, target: float) -> float:
    if maxabs <= 0:
        return 1.0
    return float(2.0 ** np.floor(np.log2(target / maxabs)))


def host_prepare(inputs: dict) -> tuple[dict, dict, dict]:
    """Quantize weights/x to fp8, fold dropout scaling + per-layer scales,
    compute masks, tile/shard x and masks.

    Returns (shared_inputs, per_core_varying, act_scale).
    """
    import jax
    import ml_dtypes

    E4 = ml_dtypes.float8_e4m3  # TRN FP8_EXP4: bias 7, max normal +-240

    x = np.asarray(inputs["x"], dtype=np.float32)
    W = {i: np.asarray(inputs[f"W{i}"], dtype=np.float32) for i in range(1, 9)}
    b = {i: np.asarray(inputs[f"b{i}"], dtype=np.float32) for i in range(1, 9)}

    # Dropout masks — bit-exact replication of the reference's PRNG stream.
    cpu = jax.devices("cpu")[0]
    with jax.default_device(cpu):
        dk = jax.random.split(jax.random.key(42), 3)
        keeps = {
            l: np.asarray(
                jax.random.bernoulli(dk[i], KEEP[l], (BATCH, H)), dtype=np.uint8
            )
            for i, l in enumerate(DROP_LAYERS)
        }

    # Fold 1/(1-p) into the next layer's weights.
    Wf = dict(W)
    for l in DROP_LAYERS:
        Wf[l + 1] = (W[l + 1] / np.float32(KEEP[l])).astype(np.float32)

    # Per-tensor power-of-2 fp8 scales. Weight maxes are exact; activation
    # maxes are estimated from a 2048-row fp32 forward pass with ~2.5x
    # headroom to the 240 clip.
    beta = {l: _pow2_scale(float(np.abs(Wf[l]).max()), 160.0) for l in range(1, 9)}
    alpha = {0: _pow2_scale(float(np.abs(x).max()), 160.0)}
    sub = x[:2048]
    hs = sub
    for l in range(1, 8):
        z = hs @ Wf[l] + b[l]
        hs = np.maximum(z, 0.0)
        if l in DROP_LAYERS:
            hs = hs * keeps[l][:2048] / np.float32(KEEP[l])
        alpha[l] = _pow2_scale(float(hs.max()), 96.0)

    def q8(a, scale):
        return np.clip(a * np.float32(scale), -240.0, 240.0).astype(E4)

    # Weights in DoubleRow layout [128, KG, 2, N]:
    # element (p, kg, i, n) = Wq[kg*256 + i*128 + p, n].
    W1p = np.zeros((D_PAD, H), dtype=np.float32)
    W1p[:D_IN] = Wf[1]
    W8p = np.zeros((H, CP), dtype=np.float32)
    W8p[:, :C] = Wf[8]

    def dr_layout(wq):
        kin, n = wq.shape
        return np.ascontiguousarray(
            wq.reshape(kin // 256, 2, 128, n).transpose(2, 0, 1, 3)
        )

    shared = {
        "w1": dr_layout(q8(W1p, beta[1])),
        "w8": dr_layout(q8(W8p, beta[8])),
    }
    for l in range(2, 8):
        shared[f"w{l}"] = dr_layout(q8(Wf[l], beta[l]))

    bias17 = np.empty((128, 28), dtype=np.float32)
    for l in range(1, 8):
        bias17[:, (l - 1) * 4 : l * 4] = (alpha[l] * b[l]).reshape(4, 128).T
    b8c = np.zeros((128, 1), dtype=np.float32)
    b8c[:C, 0] = b[8]
    shared["bias17"] = bias17
    shared["b8c"] = b8c

    act_scale = {l: alpha[l] / (alpha[l - 1] * beta[l]) for l in range(1, 8)}
    act_scale[8] = 1.0 / (alpha[7] * beta[8])

    # x: quantize, pad to 1024 features, tile to [128, nbt, 8, BT] per core
    # so each batch-tile DMA reads 4KB contiguous per partition.
    xq = np.zeros((BATCH, D_PAD), dtype=E4)
    xq[:, :D_IN] = q8(x, alpha[0])
    xb = xq.view(np.uint8)
    nbt = B_CORE // BT
    per_core = {"xT": [], "m2": [], "m4": [], "m6": []}
    for c in range(N_CORES):
        sl = slice(c * B_CORE, (c + 1) * B_CORE)
        slab = xb[sl].reshape(nbt, BT, 2 * KG1, 128).transpose(3, 0, 2, 1)
        per_core["xT"].append(np.ascontiguousarray(slab).view(E4))
        for l in DROP_LAYERS:
            mslab = keeps[l][sl].reshape(nbt, BT, KO, 128).transpose(3, 0, 2, 1)
            per_core[f"m{l}"].append(np.ascontiguousarray(mslab))
    return shared, per_core, act_scale


def run_hw(inputs: dict, trace: bool = False):
    from concourse import bass_utils

    shared, per_core, act_scale = host_prepare(inputs)
    nc = build_bass(B_CORE, act_scale)
    in_maps = [
        {**shared, **{k: v[c] for k, v in per_core.items()}} for c in range(N_CORES)
    ]
    res = bass_utils.run_bass_kernel_spmd(
        nc, in_maps, core_ids=list(range(N_CORES)), trace=trace
    )
    out = np.concatenate([np.ascontiguousarray(r["yT"].T) for r in res.results], axis=0)
    return out.astype(np.float32), res


def kernel(**inputs) -> np.ndarray:
    return run_hw(inputs, trace=False)[0]


# revision 27
# speedup vs baseline: 1.0320x; 1.0320x over previous
"""Trainium2 Bass kernel for an 8-layer dense MLP (784->512x6->10) + softmax.

Strategy (hardcoded for batch=65536, 8 NeuronCores, pure data parallel):
  - Each core handles 8192 rows of the batch; weights replicated.
  - All matmuls run in fp8-e4m3 with perf_mode=DoubleRow: the PE holds two
    fp8 weight planes per cell, contracting 256 features per matmul at
    2 MACs/cell/cycle (~2x the fp32r column rate). PSUM accumulates fp32.
  - Per-tensor power-of-2 scales keep fp8 ranges centered: weights scaled by
    beta_l, activations by alpha_l (estimated from a host-side subsample
    forward pass). The rescale alpha_l/(alpha_{l-1} beta_l) and the bias are
    folded into the PSUM-draining op.
  - PSUM drains are split across engines to balance throughput (ACT does a
    [128,512] relu-drain in ~720ns, the DVE pair mult-add + relu-cast in
    ~1.1us, and the PE issues a DoubleRow matmul every ~216ns): 19 of the 28
    per-tile drains go to ACT, 9 to DVE; all 12 dropout mask-multiplies go
    to GpSimd.
  - Dropout masks (jax threefry, key 42) are bit-exactly precomputed on host
    and shipped as {0,1} uint8 masks; 1/(1-p) is folded into the next layer's
    weights before quantization.
  - Softmax: exp on ACT (bias=b8, scale=1/(alpha7 beta8)), class sums via a
    10x10 ones-matmul on the PE (replicated across the 10 partitions in
    PSUM), reciprocal + multiply on DVE. No max-subtraction (logits are
    O(0.1); exp is safe in fp32).
"""

import numpy as np

BATCH = 65536
D_IN = 784
D_PAD = 1024               # padded input features: 4 DoubleRow groups of 256
KG1 = 4                    # layer-1 DoubleRow k-groups
H = 512
KO = 4                     # 128-feature chunks per hidden activation
KG = 2                     # hidden-layer DoubleRow k-groups (2 x 256 = 512)
C = 10
CP = 16                    # layer-8 output padded to 16 for AP alignment
N_CORES = 8
B_CORE = BATCH // N_CORES  # 8192
BT = 512                   # batch tile (matmul moving free dim)

DROP_LAYERS = (2, 4, 6)    # dropout applied to these layers' outputs
KEEP = {2: 0.8, 4: 0.7, 6: 0.5}

# Drain-engine split: chunk n of layer l drains on DVE iff listed here.
# (Measured: ACT does a [128,512] drain in one ~720ns op; a DVE drain needs
# two tensor_scalar ops at ~1.9us total. ACT saturates at ~22 drains/tile,
# so 6 late chunks of the non-dropout layers go to DVE; the pair-wavefront
# below hides their latency behind the partner tile's matmuls.)
DVE_DRAINS = {(1, 2), (1, 3), (3, 2), (3, 3), (5, 3), (7, 3)}


def build_bass(b_core: int, act_scale: dict):
    """Build the Bass module for one core processing b_core batch rows.

    act_scale maps layer -> float scale applied inside the PSUM-draining
    op (alpha_l / (alpha_{l-1} beta_l) for hidden, 1/(alpha7 beta8) for
    the logits).
    """
    import concourse.mybir as mybir
    import concourse.tile as tile
    from concourse import bacc

    f32 = mybir.dt.float32
    f32r = mybir.dt.float32r
    f8 = mybir.dt.float8e4
    u8 = mybir.dt.uint8
    AF = mybir.ActivationFunctionType
    ALU = mybir.AluOpType
    DR = mybir.MatmulPerfMode.DoubleRow

    nbt = b_core // BT

    nc = bacc.Bacc("TRN2", target_bir_lowering=False, debug=False)

    xT = nc.dram_tensor("xT", [128, nbt, 2 * KG1, BT], f8, kind="ExternalInput")
    w_h = {1: nc.dram_tensor("w1", [128, KG1, 2, H], f8, kind="ExternalInput")}
    for l in range(2, 8):
        w_h[l] = nc.dram_tensor(f"w{l}", [128, KG, 2, H], f8, kind="ExternalInput")
    w8_h = nc.dram_tensor("w8", [128, KG, 2, CP], f8, kind="ExternalInput")
    bias17_h = nc.dram_tensor("bias17", [128, 28], f32, kind="ExternalInput")
    b8c_h = nc.dram_tensor("b8c", [128, 1], f32, kind="ExternalInput")
    m_h = {
        l: nc.dram_tensor(f"m{l}", [128, nbt, KO, BT], u8, kind="ExternalInput")
        for l in DROP_LAYERS
    }
    y_h = nc.dram_tensor("yT", [C, b_core], f32, kind="ExternalOutput")

    with tile.TileContext(nc) as tc:
        with (
            tc.tile_pool(name="wpool", bufs=1) as wpool,
            tc.tile_pool(name="xpool", bufs=4) as xpool,
            tc.tile_pool(name="hpool", bufs=5) as hpool,
            tc.tile_pool(name="tpool", bufs=3) as tpool,
            tc.tile_pool(name="mpool", bufs=4) as mpool,
            tc.tile_pool(name="spool", bufs=4) as spool,
            tc.tile_pool(name="psum", bufs=4, space="PSUM") as pp,
            tc.tile_pool(name="psum8", bufs=2, space="PSUM") as pp8,
            tc.tile_pool(name="psums", bufs=2, space="PSUM") as pps,
        ):
            gate = {"inst": None}
            chain = {"prev": None}

            def chained(di):
                if chain["prev"] is not None:
                    tile.add_dep_helper(di.ins, chain["prev"].ins, sync=True)
                chain["prev"] = di
                return di

            def load_bt(bt, in_chain=False):
                xt = xpool.tile([128, 2 * KG1, BT], f8, tag="xt", name="xt")
                di = nc.sync.dma_start(xt[:], xT.ap()[:, bt, :, :])
                if in_chain:
                    chained(di)
                if gate["inst"] is not None:
                    # Prefetches for bt>=2 may not be hoisted ahead of the
                    # weight stream: gate them on the last hidden weight DMA.
                    tile.add_dep_helper(di.ins, gate["inst"], sync=True)
                mt = {}
                for l in DROP_LAYERS:
                    mt[l] = mpool.tile([128, KO, BT], u8, tag=f"m{l}", name=f"m{l}_t")
                    mi = nc.sync.dma_start(mt[l][:], m_h[l].ap()[:, bt, :, :])
                    if gate["inst"] is not None:
                        tile.add_dep_helper(mi.ins, gate["inst"], sync=True)
                return xt, mt

            # Warm the PE HAM clock-gate with dummy fp32 matmuls that run
            # during the initial DMA wait (~3.4us of activity -> K=8/8).
            warm_w = wpool.tile([128, 128], f32, tag="warm_w")
            warm_x = wpool.tile([128, BT], f32, tag="warm_x")
            nc.vector.memset(warm_w[:], 0)
            nc.vector.memset(warm_x[:], 0)
            ones10 = wpool.tile([C, C], f32, tag="ones10")
            nc.vector.memset(ones10[:], 1.0)
            warm_ps = pp.tile([128, BT], f32, tag="ps", name="warm_ps")
            for _ in range(7):
                nc.tensor.matmul(warm_ps[:], lhsT=warm_w[:], rhs=warm_x[:])

            # Startup DMAs are chained into a forced serial order
            # xt0 -> w1 -> xt1 -> w2..w7 so each transfer gets the full queue
            # bandwidth and the scheduler cannot hoist prefetches ahead of the
            # weight stream; the two-tile wavefront below consumes them in
            # exactly this order.
            xt0, mt0 = load_bt(0, in_chain=True)
            w_t = {1: wpool.tile([128, KG1, 2, H], f8, tag="w1", name="w1_t")}
            chained(nc.sync.dma_start(w_t[1][:], w_h[1].ap()))
            xt1, mt1 = load_bt(1, in_chain=True)
            w7_dma = None
            for l in range(2, 8):
                w_t[l] = wpool.tile([128, KG, 2, H], f8, tag=f"w{l}", name=f"w{l}_t")
                w7_dma = chained(nc.sync.dma_start(w_t[l][:], w_h[l].ap()))
            w8_t = wpool.tile([128, KG, 2, CP], f8, tag="w8")
            nc.sync.dma_start(w8_t[:], w8_h.ap())
            bias17_t = wpool.tile([128, 28], f32, tag="bias17")
            nc.sync.dma_start(bias17_t[:], bias17_h.ap())
            b8c_t = wpool.tile([128, 1], f32, tag="b8c")
            nc.sync.dma_start(b8c_t[:], b8c_h.ap())
            gate["inst"] = w7_dma.ins

            def hidden_layer(l, src, mt, is_a=True):
                kg_in = KG1 if l == 1 else KG
                hn = hpool.tile([128, KO, BT], f8, tag="h", name="h")
                # kg-outer emission: all kg=0 matmuls for the 4 n-chunks
                # first, then kg=1, ... — the chunks feeding the NEXT
                # layer's kg=1 groups are thereby needed ~900ns later,
                # hiding drain/mask latency.
                pss = [pp.tile([128, BT], f32, tag="ps", name="ps") for _ in range(KO)]
                for kg in range(kg_in):
                    for n in range(KO):
                        nc.tensor.matmul(
                            pss[n][:],
                            lhsT=w_t[l][:, kg, :, n * 128 : (n + 1) * 128],
                            rhs=src[:, 2 * kg : 2 * kg + 2, :],
                            start=(kg == 0),
                            stop=(kg == kg_in - 1),
                            perf_mode=DR,
                        )
                for n in range(KO):
                    ps = pss[n]
                    bias_ap = bias17_t[:, (l - 1) * 4 + n : (l - 1) * 4 + n + 1]
                    if (l, n) in DVE_DRAINS:
                        # two-op DVE drain: (scale*ps + bias) then relu+fp8
                        t32 = tpool.tile([128, BT], f32, tag="t32", name="t32")
                        nc.vector.tensor_scalar(
                            t32[:], ps[:], act_scale[l], scalar2=bias_ap,
                            op0=ALU.mult, op1=ALU.add,
                        )
                        nc.vector.tensor_scalar(
                            hn[:, n, :], t32[:], 0.0, scalar2=None, op0=ALU.max,
                        )
                    else:
                        # relu(scale*psum + bias) fused, PSUM -> fp8 SBUF
                        nc.scalar.activation(
                            hn[:, n, :], ps[:], AF.Relu,
                            bias=bias_ap, scale=act_scale[l],
                        )
                    if l in DROP_LAYERS:
                        # gpsimd's tensor_tensor is slow/erratic in situ
                        # (1.4-3.8us): give it only tile A's chunk 3 — the
                        # single mask with more than a pair-layer of slack.
                        # DVE handles the rest at 602ns without saturating.
                        eng = nc.gpsimd if (n == 3 and is_a) else nc.vector
                        eng.tensor_tensor(
                            hn[:, n, :], hn[:, n, :], mt[l][:, n, :], ALU.mult
                        )
                return hn

            ot_all = wpool.tile([C, b_core], f32, tag="ot_all")

            def final_logits(h):
                # layer 8 (512->10 padded to 16), feature-major out, then
                # exp (bias=b8, scale=1/(a7 b8)) on ACT.
                ps8 = pp8.tile([CP, BT], f32, tag="ps8", name="ps8")
                for kg in range(KG):
                    nc.tensor.matmul(
                        ps8[:],
                        lhsT=w8_t[:, kg, :, :],
                        rhs=h[:, 2 * kg : 2 * kg + 2, :],
                        start=(kg == 0),
                        stop=(kg == KG - 1),
                        perf_mode=DR,
                    )
                ex = spool.tile([C, BT], f32r, tag="ex", name="ex")
                nc.scalar.activation(
                    ex[:], ps8[:C, :], AF.Exp, bias=b8c_t[:C, 0:1],
                    scale=act_scale[8],
                )
                return ex

            def final_tail(ex, bs):
                # Softmax tail, deferred one pair so the 3.3us reciprocals
                # land in the DVE queue behind the NEXT pair's drains instead
                # of ahead of them: class sums replicated via a 10x10
                # ones-matmul on PE, reciprocal + multiply on DVE.
                ps_s = pps.tile([C, BT], f32, tag="ps_s", name="ps_s")
                nc.tensor.matmul(ps_s[:], lhsT=ones10[:].bitcast(f32r), rhs=ex[:])
                rsum = spool.tile([C, BT], f32, tag="rsum", name="rsum")
                nc.vector.reciprocal(rsum[:], ps_s[:])
                # Accumulate normalized outputs in SBUF; one DMA at the end
                # so no per-tile queue ever blocks on the softmax tail.
                nc.vector.tensor_tensor(
                    ot_all[:, bs : bs + BT], ex[:], rsum[:], ALU.mult
                )

            # All tiles run as two-tile wavefront pairs: while tile A's layer
            # drains complete on ACT/DVE, the PE runs tile B's matmuls for
            # the same layer, hiding drain latency. Pair 0 additionally
            # covers the startup weight-stream window.
            pending = None
            preloaded = None
            for pair in range(0, nbt, 2):
                if pair == 0:
                    curA, mtA, curB, mtB = xt0, mt0, xt1, mt1
                else:
                    curA, mtA, curB, mtB = preloaded
                for l in range(1, 8):
                    curA = hidden_layer(l, curA, mtA, is_a=True)
                    curB = hidden_layer(l, curB, mtB, is_a=False)
                    if l == 2 and pair + 2 < nbt:
                        # Prefetch the next pair's x/mask tiles a full pair
                        # ahead so L1 never waits on the inbound DMA.
                        nxA, nmA = load_bt(pair + 2)
                        nxB, nmB = load_bt(pair + 3)
                        preloaded = (nxA, nmA, nxB, nmB)
                if pending is not None:
                    final_tail(pending[0], pending[1])
                    final_tail(pending[2], pending[3])
                pending = (
                    final_logits(curA), pair * BT,
                    final_logits(curB), (pair + 1) * BT,
                )
            final_tail(pending[0], pending[1])
            final_tail(pending[2], pending[3])

            nc.sync.dma_start(y_h.ap(), ot_all[:])

    nc.compile()
    return nc


def _pow2_scale(maxabs: float, target: float) -> float:
    if maxabs <= 0:
        return 1.0
    return float(2.0 ** np.floor(np.log2(target / maxabs)))


def host_prepare(inputs: dict) -> tuple[dict, dict, dict]:
    """Quantize weights/x to fp8, fold dropout scaling + per-layer scales,
    compute masks, tile/shard x and masks.

    Returns (shared_inputs, per_core_varying, act_scale).
    """
    import jax
    import ml_dtypes

    E4 = ml_dtypes.float8_e4m3  # TRN FP8_EXP4: bias 7, max normal +-240

    x = np.asarray(inputs["x"], dtype=np.float32)
    W = {i: np.asarray(inputs[f"W{i}"], dtype=np.float32) for i in range(1, 9)}
    b = {i: np.asarray(inputs[f"b{i}"], dtype=np.float32) for i in range(1, 9)}

    # Dropout masks — bit-exact replication of the reference's PRNG stream.
    cpu = jax.devices("cpu")[0]
    with jax.default_device(cpu):
        dk = jax.random.split(jax.random.key(42), 3)
        keeps = {
            l: np.asarray(
                jax.random.bernoulli(dk[i], KEEP[l], (BATCH, H)), dtype=np.uint8
            )
            for i, l in enumerate(DROP_LAYERS)
        }

    # Fold 1/(1-p) into the next layer's weights.
    Wf = dict(W)
    for l in DROP_LAYERS:
        Wf[l + 1] = (W[l + 1] / np.float32(KEEP[l])).astype(np.float32)

    # Per-tensor power-of-2 fp8 scales. Weight maxes are exact; activation
    # maxes are estimated from a 2048-row fp32 forward pass with ~2.5x
    # headroom to the 240 clip.
    beta = {l: _pow2_scale(float(np.abs(Wf[l]).max()), 160.0) for l in range(1, 9)}
    alpha = {0: _pow2_scale(float(np.abs(x).max()), 160.0)}
    sub = x[:2048]
    hs = sub
    for l in range(1, 8):
        z = hs @ Wf[l] + b[l]
        hs = np.maximum(z, 0.0)
        if l in DROP_LAYERS:
            hs = hs * keeps[l][:2048] / np.float32(KEEP[l])
        alpha[l] = _pow2_scale(float(hs.max()), 96.0)

    def q8(a, scale):
        return np.clip(a * np.float32(scale), -240.0, 240.0).astype(E4)

    # Weights in DoubleRow layout [128, KG, 2, N]:
    # element (p, kg, i, n) = Wq[kg*256 + i*128 + p, n].
    W1p = np.zeros((D_PAD, H), dtype=np.float32)
    W1p[:D_IN] = Wf[1]
    W8p = np.zeros((H, CP), dtype=np.float32)
    W8p[:, :C] = Wf[8]

    def dr_layout(wq):
        kin, n = wq.shape
        return np.ascontiguousarray(
            wq.reshape(kin // 256, 2, 128, n).transpose(2, 0, 1, 3)
        )

    shared = {
        "w1": dr_layout(q8(W1p, beta[1])),
        "w8": dr_layout(q8(W8p, beta[8])),
    }
    for l in range(2, 8):
        shared[f"w{l}"] = dr_layout(q8(Wf[l], beta[l]))

    bias17 = np.empty((128, 28), dtype=np.float32)
    for l in range(1, 8):
        bias17[:, (l - 1) * 4 : l * 4] = (alpha[l] * b[l]).reshape(4, 128).T
    b8c = np.zeros((128, 1), dtype=np.float32)
    b8c[:C, 0] = b[8]
    shared["bias17"] = bias17
    shared["b8c"] = b8c

    act_scale = {l: alpha[l] / (alpha[l - 1] * beta[l]) for l in range(1, 8)}
    act_scale[8] = 1.0 / (alpha[7] * beta[8])

    # x: quantize, pad to 1024 features, tile to [128, nbt, 8, BT] per core
    # so each batch-tile DMA reads 4KB contiguous per partition.
    xq = np.zeros((BATCH, D_PAD), dtype=E4)
    xq[:, :D_IN] = q8(x, alpha[0])
    xb = xq.view(np.uint8)
    nbt = B_CORE // BT
    per_core = {"xT": [], "m2": [], "m4": [], "m6": []}
    for c in range(N_CORES):
        sl = slice(c * B_CORE, (c + 1) * B_CORE)
        slab = xb[sl].reshape(nbt, BT, 2 * KG1, 128).transpose(3, 0, 2, 1)
        per_core["xT"].append(np.ascontiguousarray(slab).view(E4))
        for l in DROP_LAYERS:
            mslab = keeps[l][sl].reshape(nbt, BT, KO, 128).transpose(3, 0, 2, 1)
            per_core[f"m{l}"].append(np.ascontiguousarray(mslab))
    return shared, per_core, act_scale


def run_hw(inputs: dict, trace: bool = False):
    from concourse import bass_utils

    shared, per_core, act_scale = host_prepare(inputs)
    nc = build_bass(B_CORE, act_scale)
    in_maps = [
        {**shared, **{k: v[c] for k, v in per_core.items()}} for c in range(N_CORES)
    ]
    res = bass_utils.run_bass_kernel_spmd(
        nc, in_maps, core_ids=list(range(N_CORES)), trace=trace
    )
    out = np.concatenate([np.ascontiguousarray(r["yT"].T) for r in res.results], axis=0)
    return out.astype(np.float32), res


def kernel(**inputs) -> np.ndarray:
    return run_hw(inputs, trace=False)[0]


# revision 28
# speedup vs baseline: 1.1029x; 1.0687x over previous
"""Trainium2 Bass kernel for an 8-layer dense MLP (784->512x6->10) + softmax.

Strategy (hardcoded for batch=65536, 8 NeuronCores, pure data parallel):
  - Each core handles 8192 rows of the batch; weights replicated.
  - All matmuls run in fp8-e4m3 with perf_mode=DoubleRow: the PE holds two
    fp8 weight planes per cell, contracting 256 features per matmul at
    2 MACs/cell/cycle (~2x the fp32r column rate). PSUM accumulates fp32.
  - Per-tensor power-of-2 scales keep fp8 ranges centered: weights scaled by
    beta_l, activations by alpha_l (estimated from a host-side subsample
    forward pass). The rescale alpha_l/(alpha_{l-1} beta_l) and the bias are
    folded into the PSUM-draining op.
  - PSUM drains are split across engines to balance throughput (ACT does a
    [128,512] relu-drain in ~720ns, the DVE pair mult-add + relu-cast in
    ~1.1us, and the PE issues a DoubleRow matmul every ~216ns): 19 of the 28
    per-tile drains go to ACT, 9 to DVE; all 12 dropout mask-multiplies go
    to GpSimd.
  - Dropout masks (jax threefry, key 42) are bit-exactly precomputed on host
    and shipped as {0,1} uint8 masks; 1/(1-p) is folded into the next layer's
    weights before quantization.
  - Softmax: exp on ACT (bias=b8, scale=1/(alpha7 beta8)), class sums via a
    10x10 ones-matmul on the PE (replicated across the 10 partitions in
    PSUM), reciprocal + multiply on DVE. No max-subtraction (logits are
    O(0.1); exp is safe in fp32).
"""

import numpy as np

BATCH = 65536
D_IN = 784
D_PAD = 1024               # padded input features: 4 DoubleRow groups of 256
KG1 = 4                    # layer-1 DoubleRow k-groups
H = 512
KO = 4                     # 128-feature chunks per hidden activation
KG = 2                     # hidden-layer DoubleRow k-groups (2 x 256 = 512)
C = 10
CP = 16                    # layer-8 output padded to 16 for AP alignment
N_CORES = 8
B_CORE = BATCH // N_CORES  # 8192
BT = 512                   # batch tile (matmul moving free dim)

DROP_LAYERS = (2, 4, 6)    # dropout applied to these layers' outputs
KEEP = {2: 0.8, 4: 0.7, 6: 0.5}

# Drain-engine split: chunk n of layer l drains on DVE iff listed here.
# (Measured: ACT does a [128,512] drain in one ~720ns op; a DVE drain needs
# two tensor_scalar ops at ~1.9us total. ACT saturates at ~22 drains/tile,
# so 6 late chunks of the non-dropout layers go to DVE; the pair-wavefront
# below hides their latency behind the partner tile's matmuls.)
DVE_DRAINS = {(1, 2), (1, 3), (3, 2), (3, 3), (5, 3), (7, 3)}


def build_bass(b_core: int, act_scale: dict):
    """Build the Bass module for one core processing b_core batch rows.

    act_scale maps layer -> float scale applied inside the PSUM-draining
    op (alpha_l / (alpha_{l-1} beta_l) for hidden, 1/(alpha7 beta8) for
    the logits).
    """
    import concourse.mybir as mybir
    import concourse.tile as tile
    from concourse import bacc

    f32 = mybir.dt.float32
    f32r = mybir.dt.float32r
    f8 = mybir.dt.float8e4
    u8 = mybir.dt.uint8
    AF = mybir.ActivationFunctionType
    ALU = mybir.AluOpType
    DR = mybir.MatmulPerfMode.DoubleRow

    nbt = b_core // BT

    nc = bacc.Bacc("TRN2", target_bir_lowering=False, debug=False)

    xT = nc.dram_tensor("xT", [128, nbt, 2 * KG1, BT], f8, kind="ExternalInput")
    w_h = {1: nc.dram_tensor("w1", [128, KG1, 2, H], f8, kind="ExternalInput")}
    for l in range(2, 8):
        w_h[l] = nc.dram_tensor(f"w{l}", [128, KG, 2, H], f8, kind="ExternalInput")
    w8_h = nc.dram_tensor("w8", [128, KG, 2, CP], f8, kind="ExternalInput")
    bias17_h = nc.dram_tensor("bias17", [128, 28], f32, kind="ExternalInput")
    b8c_h = nc.dram_tensor("b8c", [128, 1], f32, kind="ExternalInput")
    m_h = {
        l: nc.dram_tensor(f"m{l}", [128, nbt, KO, BT], u8, kind="ExternalInput")
        for l in DROP_LAYERS
    }
    y_h = nc.dram_tensor("yT", [C, b_core], f32, kind="ExternalOutput")

    with tile.TileContext(nc) as tc:
        with (
            tc.tile_pool(name="wpool", bufs=1) as wpool,
            tc.tile_pool(name="xpool", bufs=4) as xpool,
            tc.tile_pool(name="hpool", bufs=5) as hpool,
            tc.tile_pool(name="tpool", bufs=3) as tpool,
            tc.tile_pool(name="mpool", bufs=4) as mpool,
            tc.tile_pool(name="spool", bufs=4) as spool,
            tc.tile_pool(name="psum", bufs=4, space="PSUM") as pp,
            tc.tile_pool(name="psum8", bufs=2, space="PSUM") as pp8,
            tc.tile_pool(name="psums", bufs=2, space="PSUM") as pps,
        ):
            gate = {"inst": None}
            chain = {"prev": None}

            def chained(di):
                if chain["prev"] is not None:
                    tile.add_dep_helper(di.ins, chain["prev"].ins, sync=True)
                chain["prev"] = di
                return di

            def load_bt(bt, in_chain=False):
                xt = xpool.tile([128, 2 * KG1, BT], f8, tag="xt", name="xt")
                di = nc.sync.dma_start(xt[:], xT.ap()[:, bt, :, :])
                if in_chain:
                    chained(di)
                if gate["inst"] is not None:
                    # Prefetches for bt>=2 may not be hoisted ahead of the
                    # weight stream: gate them on the last hidden weight DMA.
                    tile.add_dep_helper(di.ins, gate["inst"], sync=True)
                mt = {}
                for l in DROP_LAYERS:
                    mt[l] = mpool.tile([128, KO, BT], u8, tag=f"m{l}", name=f"m{l}_t")
                    mi = nc.sync.dma_start(mt[l][:], m_h[l].ap()[:, bt, :, :])
                    if gate["inst"] is not None:
                        tile.add_dep_helper(mi.ins, gate["inst"], sync=True)
                return xt, mt

            # Warm the PE HAM clock-gate with dummy fp32 matmuls that run
            # during the initial DMA wait (~3.4us of activity -> K=8/8).
            warm_w = wpool.tile([128, 128], f32, tag="warm_w")
            warm_x = wpool.tile([128, BT], f32, tag="warm_x")
            nc.vector.memset(warm_w[:], 0)
            nc.vector.memset(warm_x[:], 0)
            ones10 = wpool.tile([C, C], f32, tag="ones10")
            nc.vector.memset(ones10[:], 1.0)
            warm_ps = pp.tile([128, BT], f32, tag="ps", name="warm_ps")
            for _ in range(7):
                nc.tensor.matmul(warm_ps[:], lhsT=warm_w[:], rhs=warm_x[:])

            # Startup DMAs are chained into a forced serial order
            # xt0 -> w1 -> xt1 -> w2..w7 so each transfer gets the full queue
            # bandwidth and the scheduler cannot hoist prefetches ahead of the
            # weight stream; the two-tile wavefront below consumes them in
            # exactly this order.
            xt0, mt0 = load_bt(0, in_chain=True)
            w_t = {1: wpool.tile([128, KG1, 2, H], f8, tag="w1", name="w1_t")}
            chained(nc.sync.dma_start(w_t[1][:], w_h[1].ap()))
            xt1, mt1 = load_bt(1, in_chain=True)
            w7_dma = None
            for l in range(2, 8):
                w_t[l] = wpool.tile([128, KG, 2, H], f8, tag=f"w{l}", name=f"w{l}_t")
                w7_dma = chained(nc.sync.dma_start(w_t[l][:], w_h[l].ap()))
            w8_t = wpool.tile([128, KG, 2, CP], f8, tag="w8")
            nc.sync.dma_start(w8_t[:], w8_h.ap())
            bias17_t = wpool.tile([128, 28], f32, tag="bias17")
            nc.sync.dma_start(bias17_t[:], bias17_h.ap())
            b8c_t = wpool.tile([128, 1], f32, tag="b8c")
            nc.sync.dma_start(b8c_t[:], b8c_h.ap())
            gate["inst"] = w7_dma.ins

            def hidden_layer(l, src, mt, is_a=True):
                kg_in = KG1 if l == 1 else KG
                hn = hpool.tile([128, KO, BT], f8, tag="h", name="h")
                # kg-outer emission: all kg=0 matmuls for the 4 n-chunks
                # first, then kg=1, ... — the chunks feeding the NEXT
                # layer's kg=1 groups are thereby needed ~900ns later,
                # hiding drain/mask latency.
                pss = [pp.tile([128, BT], f32, tag="ps", name="ps") for _ in range(KO)]
                for kg in range(kg_in):
                    for n in range(KO):
                        nc.tensor.matmul(
                            pss[n][:],
                            lhsT=w_t[l][:, kg, :, n * 128 : (n + 1) * 128],
                            rhs=src[:, 2 * kg : 2 * kg + 2, :],
                            start=(kg == 0),
                            stop=(kg == kg_in - 1),
                            perf_mode=DR,
                        )
                for n in range(KO):
                    ps = pss[n]
                    bias_ap = bias17_t[:, (l - 1) * 4 + n : (l - 1) * 4 + n + 1]
                    if (l, n) in DVE_DRAINS:
                        # two-op DVE drain: (scale*ps + bias) then relu+fp8
                        t32 = tpool.tile([128, BT], f32, tag="t32", name="t32")
                        nc.vector.tensor_scalar(
                            t32[:], ps[:], act_scale[l], scalar2=bias_ap,
                            op0=ALU.mult, op1=ALU.add,
                        )
                        nc.vector.tensor_scalar(
                            hn[:, n, :], t32[:], 0.0, scalar2=None, op0=ALU.max,
                        )
                    else:
                        # relu(scale*psum + bias) fused, PSUM -> fp8 SBUF
                        nc.scalar.activation(
                            hn[:, n, :], ps[:], AF.Relu,
                            bias=bias_ap, scale=act_scale[l],
                        )
                    if l in DROP_LAYERS:
                        # gpsimd's tensor_tensor is slow/erratic in situ
                        # (1.4-3.8us): give it only chunk 3, which the next
                        # layer needs last; DVE masks the rest at 602ns.
                        # (Measured: moving more masks either way is slower.)
                        eng = nc.vector if n <= 2 else nc.gpsimd
                        eng.tensor_tensor(
                            hn[:, n, :], hn[:, n, :], mt[l][:, n, :], ALU.mult
                        )
                return hn

            ot_all = wpool.tile([C, b_core], f32, tag="ot_all")

            def final_logits(h):
                # layer 8 (512->10 padded to 16), feature-major out, then
                # exp (bias=b8, scale=1/(a7 b8)) on ACT.
                ps8 = pp8.tile([CP, BT], f32, tag="ps8", name="ps8")
                for kg in range(KG):
                    nc.tensor.matmul(
                        ps8[:],
                        lhsT=w8_t[:, kg, :, :],
                        rhs=h[:, 2 * kg : 2 * kg + 2, :],
                        start=(kg == 0),
                        stop=(kg == KG - 1),
                        perf_mode=DR,
                    )
                ex = spool.tile([C, BT], f32r, tag="ex", name="ex")
                nc.scalar.activation(
                    ex[:], ps8[:C, :], AF.Exp, bias=b8c_t[:C, 0:1],
                    scale=act_scale[8],
                )
                return ex

            def final_tail(ex, bs):
                # Softmax tail, deferred one pair so the 3.3us reciprocals
                # land in the DVE queue behind the NEXT pair's drains instead
                # of ahead of them: class sums replicated via a 10x10
                # ones-matmul on PE, reciprocal + multiply on DVE.
                ps_s = pps.tile([C, BT], f32, tag="ps_s", name="ps_s")
                nc.tensor.matmul(ps_s[:], lhsT=ones10[:].bitcast(f32r), rhs=ex[:])
                rsum = spool.tile([C, BT], f32, tag="rsum", name="rsum")
                nc.vector.reciprocal(rsum[:], ps_s[:])
                # Accumulate normalized outputs in SBUF; one DMA at the end
                # so no per-tile queue ever blocks on the softmax tail.
                nc.vector.tensor_tensor(
                    ot_all[:, bs : bs + BT], ex[:], rsum[:], ALU.mult
                )

            # All tiles run as two-tile wavefront pairs: while tile A's layer
            # drains complete on ACT/DVE, the PE runs tile B's matmuls for
            # the same layer, hiding drain latency. Pair 0 additionally
            # covers the startup weight-stream window.
            pending = None
            preloaded = None
            for pair in range(0, nbt, 2):
                if pair == 0:
                    curA, mtA, curB, mtB = xt0, mt0, xt1, mt1
                else:
                    curA, mtA, curB, mtB = preloaded
                for l in range(1, 8):
                    curA = hidden_layer(l, curA, mtA, is_a=True)
                    curB = hidden_layer(l, curB, mtB, is_a=False)
                    if l == 2 and pair + 2 < nbt:
                        # Prefetch the next pair's x/mask tiles a full pair
                        # ahead so L1 never waits on the inbound DMA.
                        nxA, nmA = load_bt(pair + 2)
                        nxB, nmB = load_bt(pair + 3)
                        preloaded = (nxA, nmA, nxB, nmB)
                if pending is not None:
                    final_tail(pending[0], pending[1])
                    final_tail(pending[2], pending[3])
                pending = (
                    final_logits(curA), pair * BT,
                    final_logits(curB), (pair + 1) * BT,
                )
            final_tail(pending[0], pending[1])
            final_tail(pending[2], pending[3])

            nc.sync.dma_start(y_h.ap(), ot_all[:])

    nc.compile()
    return nc


def _pow2_scale(maxabs: float, target: float) -> float:
    if maxabs <= 0:
        return 1.0
    return float(2.0 ** np.floor(np.log2(target / maxabs)))


def host_prepare(inputs: dict) -> tuple[dict, dict, dict]:
    """Quantize weights/x to fp8, fold dropout scaling + per-layer scales,
    compute masks, tile/shard x and masks.

    Returns (shared_inputs, per_core_varying, act_scale).
    """
    import jax
    import ml_dtypes

    E4 = ml_dtypes.float8_e4m3  # TRN FP8_EXP4: bias 7, max normal +-240

    x = np.asarray(inputs["x"], dtype=np.float32)
    W = {i: np.asarray(inputs[f"W{i}"], dtype=np.float32) for i in range(1, 9)}
    b = {i: np.asarray(inputs[f"b{i}"], dtype=np.float32) for i in range(1, 9)}

    # Dropout masks — bit-exact replication of the reference's PRNG stream.
    cpu = jax.devices("cpu")[0]
    with jax.default_device(cpu):
        dk = jax.random.split(jax.random.key(42), 3)
        keeps = {
            l: np.asarray(
                jax.random.bernoulli(dk[i], KEEP[l], (BATCH, H)), dtype=np.uint8
            )
            for i, l in enumerate(DROP_LAYERS)
        }

    # Fold 1/(1-p) into the next layer's weights.
    Wf = dict(W)
    for l in DROP_LAYERS:
        Wf[l + 1] = (W[l + 1] / np.float32(KEEP[l])).astype(np.float32)

    # Per-tensor power-of-2 fp8 scales. Weight maxes are exact; activation
    # maxes are estimated from a 2048-row fp32 forward pass with ~2.5x
    # headroom to the 240 clip.
    beta = {l: _pow2_scale(float(np.abs(Wf[l]).max()), 160.0) for l in range(1, 9)}
    alpha = {0: _pow2_scale(float(np.abs(x).max()), 160.0)}
    sub = x[:2048]
    hs = sub
    for l in range(1, 8):
        z = hs @ Wf[l] + b[l]
        hs = np.maximum(z, 0.0)
        if l in DROP_LAYERS:
            hs = hs * keeps[l][:2048] / np.float32(KEEP[l])
        alpha[l] = _pow2_scale(float(hs.max()), 96.0)

    def q8(a, scale):
        return np.clip(a * np.float32(scale), -240.0, 240.0).astype(E4)

    # Weights in DoubleRow layout [128, KG, 2, N]:
    # element (p, kg, i, n) = Wq[kg*256 + i*128 + p, n].
    W1p = np.zeros((D_PAD, H), dtype=np.float32)
    W1p[:D_IN] = Wf[1]
    W8p = np.zeros((H, CP), dtype=np.float32)
    W8p[:, :C] = Wf[8]

    def dr_layout(wq):
        kin, n = wq.shape
        return np.ascontiguousarray(
            wq.reshape(kin // 256, 2, 128, n).transpose(2, 0, 1, 3)
        )

    shared = {
        "w1": dr_layout(q8(W1p, beta[1])),
        "w8": dr_layout(q8(W8p, beta[8])),
    }
    for l in range(2, 8):
        shared[f"w{l}"] = dr_layout(q8(Wf[l], beta[l]))

    bias17 = np.empty((128, 28), dtype=np.float32)
    for l in range(1, 8):
        bias17[:, (l - 1) * 4 : l * 4] = (alpha[l] * b[l]).reshape(4, 128).T
    b8c = np.zeros((128, 1), dtype=np.float32)
    b8c[:C, 0] = b[8]
    shared["bias17"] = bias17
    shared["b8c"] = b8c

    act_scale = {l: alpha[l] / (alpha[l - 1] * beta[l]) for l in range(1, 8)}
    act_scale[8] = 1.0 / (alpha[7] * beta[8])

    # x: quantize, pad to 1024 features, tile to [128, nbt, 8, BT] per core
    # so each batch-tile DMA reads 4KB contiguous per partition.
    xq = np.zeros((BATCH, D_PAD), dtype=E4)
    xq[:, :D_IN] = q8(x, alpha[0])
    xb = xq.view(np.uint8)
    nbt = B_CORE // BT
    per_core = {"xT": [], "m2": [], "m4": [], "m6": []}
    for c in range(N_CORES):
        sl = slice(c * B_CORE, (c + 1) * B_CORE)
        slab = xb[sl].reshape(nbt, BT, 2 * KG1, 128).transpose(3, 0, 2, 1)
        per_core["xT"].append(np.ascontiguousarray(slab).view(E4))
        for l in DROP_LAYERS:
            mslab = keeps[l][sl].reshape(nbt, BT, KO, 128).transpose(3, 0, 2, 1)
            per_core[f"m{l}"].append(np.ascontiguousarray(mslab))
    return shared, per_core, act_scale


def run_hw(inputs: dict, trace: bool = False):
    from concourse import bass_utils

    shared, per_core, act_scale = host_prepare(inputs)
    nc = build_bass(B_CORE, act_scale)
    in_maps = [
        {**shared, **{k: v[c] for k, v in per_core.items()}} for c in range(N_CORES)
    ]
    res = bass_utils.run_bass_kernel_spmd(
        nc, in_maps, core_ids=list(range(N_CORES)), trace=trace
    )
    out = np.concatenate([np.ascontiguousarray(r["yT"].T) for r in res.results], axis=0)
    return out.astype(np.float32), res


def kernel(**inputs) -> np.ndarray:
    return run_hw(inputs, trace=False)[0]
